# revision 16
# baseline (speedup 1.0000x reference)
"""DA-RNN encoder (input-attention LSTM, softmax over BATCH dim) on TRN2.

Strategy
--------
255 sequential LSTM steps with a batch-coupled softmax make per-step
cross-core communication mandatory for any real sharding -- but the TRN2
collective floor is ~5-10us per call, i.e. >=1.3ms just in collectives for
255 steps, far more than the whole computation.  So the recurrence is run
fully REPLICATED on all 8 cores (identical SPMD graph, zero collectives)
and the per-step critical path is minimized instead:

 * softmax factorization: score[b,n,t] = score_x[b,n] + s_b(t), so
   attn = E[b,n]*e_b(t)/d_n(t) with E = exp(score_x - max_n) precomputed
   once and d_n = sum_b E[b,n] e_b(t).  The per-step softmax collapses to
   256 exps + one fused multiply-reduce.
 * everything is kept feature-major ([feature, batch] layout) so no
   transposes appear anywhere in the loop; host does the layout shuffles.
 * all matmuls run as float32r (full PE speed at free-dim>=256).
 * single ACT table set (exp_and_others: Exp + Tanh).  Sigmoid is computed
   as 0.5*tanh(x/2)+0.5, with the affine folded into fused DVE ops
   (affine_mul_reduce) and the g-gate weights pre-scaled by 2 so one
   tanh(0.5*x) ACT op covers all four gates.
"""

import os
import sys

import numpy as np

sys.path.insert(0, "/opt/trn_rl_repo")

from concourse import bacc, bass, mybir  # noqa: E402
from concourse.bass_utils import run_bass_kernel_spmd  # noqa: E402
from concourse.tile import TileContext  # noqa: E402

F32 = mybir.dt.float32
F32R = mybir.dt.float32r
ALU = mybir.AluOpType
ACTF = mybir.ActivationFunctionType

B, T1, N, H = 256, 255, 128, 256
N_CORES = 8


def build(t_steps: int = T1, debug: bool = False, reps: int = 1):
    nc = bacc.Bacc(None, target_bir_lowering=False)

    x_fm = nc.declare_dram_parameter("x_fm", [T1, N, B], F32R, isOutput=False)
    wblk = nc.declare_dram_parameter("wblk", [128, 24 * 128], F32R, isOutput=False)
    swvec = nc.declare_dram_parameter("swvec", [128, 4], F32R, isOutput=False)
    wxbc = nc.declare_dram_parameter("wxbc", [128, T1], F32, isOutput=False)
    w_out = nc.declare_dram_parameter("w_out", [T1, N, B], F32R, isOutput=True)
    h_out = nc.declare_dram_parameter("h_out", [T1, 128, 2 * B], F32R, isOutput=True)
    if debug:
        dbg_tif = nc.declare_dram_parameter(
            "dbg_tif", [t_steps, 128, 8 * B], F32, isOutput=True
        )
        dbg_c = nc.declare_dram_parameter(
            "dbg_c", [t_steps, 128, 2 * B], F32R, isOutput=True
        )
        dbg_e = nc.declare_dram_parameter(
            "dbg_e", [t_steps, 1, B], F32R, isOutput=True
        )
        dbg_g = nc.declare_dram_parameter(
            "dbg_g", [t_steps, 128, 8 * B], F32, isOutput=True
        )

    with TileContext(nc) as tc:
        with (
            tc.tile_pool(name="const", bufs=1) as cpool,
            tc.tile_pool(name="xin", bufs=4) as xpool,
            tc.tile_pool(name="work", bufs=2) as wpool,
        ):
            # ---- constants into SBUF ----
            wblk_sb = cpool.tile([128, 24 * 128], F32R)
            nc.sync.dma_start(out=wblk_sb[:, :], in_=wblk[:, :])
            sw_sb = cpool.tile([128, 4], F32R)
            nc.sync.dma_start(out=sw_sb[:, :], in_=swvec[:, :])
            ones_sb = cpool.tile([1, 128], F32R)
            nc.vector.memset(ones_sb[:, :].bitcast(F32), 1.0)

            # persistent state (feature-major: partition = H mod 128,
            # free = [h_tile(2), batch(256)])
            h_sb = cpool.tile([128, 2 * B], F32R)
            c_sb = cpool.tile([128, 2 * B], F32R)
            nc.vector.memset(h_sb[:, :].bitcast(F32), 0.0)
            nc.vector.memset(c_sb[:, :].bitcast(F32), 0.0)
            E_sb = cpool.tile([128, B], F32)  # exp(score_x - mx), feature-major
            junk = cpool.tile([128, 1], F32)

            # ---- prep: score_x^T = sum_t Wx_t * x^T_t, then E ----
            wxbc_sb = cpool.tile([128, T1], F32)
            nc.sync.dma_start(out=wxbc_sb[:, :], in_=wxbc[:, :])
            score_sb = cpool.tile([128, B], F32)
            nc.vector.memset(score_sb[:, :], 0.0)
            for t in range(T1):
                xk = xpool.tile([128, B], F32R, tag="xw")
                nc.sync.dma_start(out=xk[:, :], in_=x_fm[t, :, :])
                nc.vector.scalar_tensor_tensor(
                    out=score_sb[:, :],
                    in0=xk[:, :].bitcast(F32),
                    scalar=wxbc_sb[:, t : t + 1],
                    in1=score_sb[:, :],
                    op0=ALU.mult,
                    op1=ALU.add,
                )
            mx = cpool.tile([128, 1], F32)
            nc.vector.tensor_reduce(
                mx[:, :], score_sb[:, :], mybir.AxisListType.X, ALU.max
            )
            nmx = cpool.tile([128, 1], F32)
            nc.vector.tensor_scalar_mul(nmx[:, :], mx[:, :], -1.0)
            nc.scalar.activation(E_sb[:, :], score_sb[:, :], ACTF.Exp, bias=nmx[:, :])

            # ---- main loop ----
            with (
                tc.tile_pool(name="gates_ps", bufs=1, space="PSUM") as gpsum,
                tc.tile_pool(name="s_ps", bufs=2, space="PSUM") as spsum,
                tc.tile_pool(name="eb_ps", bufs=2, space="PSUM") as epsum,
            ):
              for _rep in range(reps):
                if reps > 1:
                    nc.vector.memset(h_sb[:, :].bitcast(F32), 0.0)
                    nc.vector.memset(c_sb[:, :].bitcast(F32), 0.0)
                for t in range(t_steps):
                    xw = xpool.tile([128, B], F32R, tag="xw")
                    nc.sync.dma_start(out=xw[:, :], in_=x_fm[t, :, :])

                    # s = h@Wh + c@Wc  -> PSUM [1, 256]
                    s_ps = spsum.tile([1, B], F32, tag="s")
                    for i, (src, col) in enumerate(
                        ((h_sb, 0), (h_sb, 1), (c_sb, 2), (c_sb, 3))
                    ):
                        nc.tensor.matmul(
                            s_ps[:, :],
                            sw_sb[:, col : col + 1],
                            src[:, (col % 2) * B : (col % 2 + 1) * B],
                            start=(i == 0),
                            stop=(i == 3),
                        )

                    gates_ps = gpsum.tile([128, 8 * B], F32, tag="g")
                    # h-dependent gate blocks first (k=1,2).  PSUM has_written
                    # clears are per-BANK (2KB = two 256-wide fp32 tiles), so
                    # start=True only on the first matmul touching each bank;
                    # the bank-mate's first write still overwrites because its
                    # has_written bits are clear.
                    for gt in range(8):
                        for k in (1, 2):
                            nc.tensor.matmul(
                                gates_ps[:, gt * B : (gt + 1) * B],
                                wblk_sb[
                                    :, (gt * 3 + k) * 128 : (gt * 3 + k + 1) * 128
                                ],
                                h_sb[:, (k - 1) * B : k * B],
                                start=(k == 1 and gt % 2 == 0),
                                stop=False,
                                skip_group_check=True,
                            )

                    # e = exp(s)
                    e_sb = wpool.tile([1, B], F32R, tag="e")
                    nc.scalar.activation(e_sb[:, :], s_ps[:, :], ACTF.Exp)

                    # broadcast e across partitions via rank-1 matmul
                    eb_ps = epsum.tile([128, B], F32, tag="eb")
                    nc.tensor.matmul(
                        eb_ps[:, :],
                        ones_sb[:, :],
                        e_sb[:, :],
                        start=True,
                        stop=True,
                    )

                    # T = E*e_bcast ; d = sum_b T   (one fused DVE op)
                    T_sb = wpool.tile([128, B], F32, tag="T")
                    d_sb = wpool.tile([128, 1], F32, tag="d")
                    nc.vector.affine_mul_reduce(
                        out=T_sb[:, :],
                        accum_out=d_sb[:, :],
                        in0=eb_ps[:, :],
                        in1=E_sb[:, :],
                        scale=1.0,
                        bias=0.0,
                    )
                    r_sb = wpool.tile([128, 1], F32, tag="r")
                    nc.vector.reciprocal_approx_fast(out=r_sb[:, :], in_=d_sb[:, :])

                    # w_in^T = (x^T * r) * T
                    win_sb = wpool.tile([128, B], F32R, tag="win")
                    nc.vector.scalar_tensor_tensor(
                        out=win_sb[:, :],
                        in0=xw[:, :].bitcast(F32),
                        scalar=r_sb[:, :],
                        in1=T_sb[:, :],
                        op0=ALU.mult,
                        op1=ALU.mult,
                    )

                    # w_in-dependent gate blocks (k=0)
                    for gt in range(8):
                        nc.tensor.matmul(
                            gates_ps[:, gt * B : (gt + 1) * B],
                            wblk_sb[:, gt * 3 * 128 : (gt * 3 + 1) * 128],
                            win_sb[:, :],
                            start=False,
                            stop=(gt % 2 == 1),
                            skip_group_check=True,
                        )

                    nc.sync.dma_start(out=w_out[t, :, :], in_=win_sb[:, :])

                    if debug:
                        nc.sync.dma_start(out=dbg_e[t, :, :], in_=e_sb[:, :])
                        gcopy = wpool.tile([128, 8 * B], F32, tag="gcopy")
                        nc.vector.tensor_copy(gcopy[:, :], gates_ps[:, :])
                        nc.sync.dma_start(out=dbg_g[t, :, :], in_=gcopy[:, :])

                    # gate nonlinearity: tanh(0.5*x) for all gates
                    # (g-gate weights pre-scaled by 2 on host)
                    tif = wpool.tile([128, 8 * B], F32, tag="tif")
                    for x in range(2):
                        nc.scalar.activation(
                            tif[:, x * 4 * B : (x + 1) * 4 * B],
                            gates_ps[:, x * 4 * B : (x + 1) * 4 * B],
                            ACTF.Tanh,
                            scale=0.5,
                        )

                    # gate slice views: [128, 2(htile), 256(b)]
                    t3 = tif[:, :].rearrange("p (x q) -> p x q", x=2)
                    c3 = c_sb[:, :].bitcast(F32).rearrange("p (x q) -> p x q", x=2)
                    h3 = h_sb[:, :].rearrange("p (x q) -> p x q", x=2)  # f32r out view

                    m1 = wpool.tile([128, 2 * B], F32, tag="m1")
                    m1v = m1[:, :].rearrange("p (x q) -> p x q", x=2)
                    m2 = wpool.tile([128, 2 * B], F32, tag="m2")
                    m2v = m2[:, :].rearrange("p (x q) -> p x q", x=2)
                    # m1 = sigmoid(i)*tanh(g) = (0.5*ti+0.5)*tg
                    nc.vector.affine_mul_reduce(
                        out=m1v,
                        accum_out=junk[:, :],
                        in0=t3[:, :, 0:B],
                        in1=t3[:, :, 2 * B : 3 * B],
                        scale=0.5,
                        bias=0.5,
                    )
                    # m2 = sigmoid(f)*c
                    nc.vector.affine_mul_reduce(
                        out=m2v,
                        accum_out=junk[:, :],
                        in0=t3[:, :, B : 2 * B],
                        in1=c3,
                        scale=0.5,
                        bias=0.5,
                    )
                    nc.vector.tensor_tensor(c_sb[:, :], m1[:, :], m2[:, :], ALU.add)

                    thc = wpool.tile([128, 2 * B], F32, tag="thc")
                    nc.scalar.activation(thc[:, :], c_sb[:, :].bitcast(F32), ACTF.Tanh)
                    thcv = thc[:, :].rearrange("p (x q) -> p x q", x=2)
                    # h = sigmoid(o)*tanh(c)
                    nc.vector.affine_mul_reduce(
                        out=h3,
                        accum_out=junk[:, :],
                        in0=t3[:, :, 3 * B : 4 * B],
                        in1=thcv,
                        scale=0.5,
                        bias=0.5,
                    )
                    nc.sync.dma_start(out=h_out[t, :, :], in_=h_sb[:, :])
                    if debug:
                        nc.sync.dma_start(out=dbg_tif[t, :, :], in_=tif[:, :])
                        nc.sync.dma_start(out=dbg_c[t, :, :], in_=c_sb[:, :])

    nc.compile()
    return nc


def _prep_inputs(input_data, attn_W, attn_b, W_ih, W_hh, b_ih, b_hh):
    x_fm = np.ascontiguousarray(np.transpose(input_data, (1, 2, 0)), dtype=np.float32)

    Wh = attn_W[0, :H].astype(np.float32)
    Wc = attn_W[0, H : 2 * H].astype(np.float32)
    Wx = attn_W[0, 2 * H :].astype(np.float32)

    swvec = np.zeros((128, 4), np.float32)
    swvec[:, 0] = Wh[:128]
    swvec[:, 1] = Wh[128:]
    swvec[:, 2] = Wc[:128]
    swvec[:, 3] = Wc[128:]

    wxbc = np.ascontiguousarray(np.broadcast_to(Wx, (128, T1)))

    bias = b_ih + b_hh
    if not np.allclose(bias, 0.0):
        raise NotImplementedError("nonzero LSTM biases not supported by this kernel")

    # gate-tile order: x0:[i0 f0 g0 o0] x1:[i1 f1 g1 o1]; g rows pre-scaled by 2
    W_cat = np.concatenate([W_ih, W_hh], axis=1).astype(np.float32)  # (1024, 384)
    wblk = np.zeros((128, 24 * 128), np.float32)
    for x in range(2):
        for gi, g in enumerate(range(4)):  # i, f, g, o
            gt = x * 4 + gi
            rows = W_cat[g * 256 + x * 128 : g * 256 + (x + 1) * 128, :]  # (128, 384)
            if g == 2:
                rows = rows * 2.0
            for k in range(3):
                blk = rows[:, k * 128 : (k + 1) * 128].T  # lhsT [K, M]
                wblk[:, (gt * 3 + k) * 128 : (gt * 3 + k + 1) * 128] = blk
    return x_fm, wblk, swvec, wxbc


_CACHE = {}


def _get_nc():
    if "nc" not in _CACHE:
        t_steps = int(os.environ.get("DARNN_T_STEPS", T1))
        debug = os.environ.get("DARNN_DEBUG", "0") == "1"
        reps = int(os.environ.get("DARNN_REPS", "1"))
        _CACHE["nc"] = build(t_steps, debug=debug, reps=reps)
        _CACHE["t_steps"] = t_steps
    return _CACHE["nc"], _CACHE["t_steps"]


def run_device(x_fm, wblk, swvec, wxbc, trace=False):
    nc, t_steps = _get_nc()
    in_map = {"x_fm": x_fm, "wblk": wblk, "swvec": swvec, "wxbc": wxbc}
    res = run_bass_kernel_spmd(
        nc, [in_map] * N_CORES, list(range(N_CORES)), trace=trace
    )
    return res, t_steps


def kernel(input_data, attn_W, attn_b, W_ih, W_hh, b_ih, b_hh):
    input_data = np.asarray(input_data, np.float32)
    attn_W = np.asarray(attn_W, np.float32)
    attn_b = np.asarray(attn_b, np.float32)
    W_ih = np.asarray(W_ih, np.float32)
    W_hh = np.asarray(W_hh, np.float32)
    b_ih = np.asarray(b_ih, np.float32)
    b_hh = np.asarray(b_hh, np.float32)

    x_fm, wblk, swvec, wxbc = _prep_inputs(
        input_data, attn_W, attn_b, W_ih, W_hh, b_ih, b_hh
    )
    res, t_steps = run_device(x_fm, wblk, swvec, wxbc)
    out = res.results[0]
    w_fm = out["w_out"]  # [T1, N, B]
    h_fm = out["h_out"]  # [T1, 128, 512]

    input_weighted = np.ascontiguousarray(np.transpose(w_fm, (2, 0, 1)))
    input_encoded = np.ascontiguousarray(
        np.transpose(h_fm.reshape(T1, 128, 2, B), (3, 0, 2, 1))
    ).reshape(B, T1, H)
    return input_weighted, input_encoded
